# revision 1
# baseline (speedup 1.0000x reference)
"""Trainium2 Bass kernel for nn_Drifter (Euler integration of Fourier drift ODE).

reference semantics:
    t = arange(0, 2001, 20)  (T=101 points)
    drift(x) = sin(x*orders) @ sin_w + cos(x*orders) @ cos_w   (orders=0..7)
    x_{n+1} = x_n + drift(x_n) * 20
    xt[B, T] = all iterates, wrapped to (-pi, pi];  t_mesh = broadcast t.

Device algorithm (per core, batch sharded 8 ways):
    Work in "turns" space u = x / (2pi).  Per harmonic k=1..7 fold
    a_k sin(k x) + b_k cos(k x) = r_k sin(2pi (k u + B_k)), so one step is
        u' = u + c0 + sum_k R_k * sin(2pi * frac_c(k*u + B_k))
    where frac_c(t) = t - round(t) in [-0.5, 0.5] (fp32 magic-constant round)
    keeps the Sin activation inside its [-pi, pi] spline domain.
    Each step's wrapped slice  2pi*(u - round(u))  is DMAed to a time-major
    [T, Bc] DRAM tensor; the host transposes to [B, T].
"""

import math

import numpy as np

B = 1048576
T = 101
NCORES = 8
BC = B // NCORES          # 131072 elements per core
P = 128
F = BC // P               # 1024
FS_ORDER = 8
DT = 20.0
TWO_PI = 2.0 * math.pi
MAGIC = 12582912.0        # 1.5 * 2**23: (x + MAGIC) - MAGIC == round(x), |x| < 2**22
_LAST_NC = None


def _build_bass(Bk, Rk, c0):
    """Bass program: in u0 [128, 1024] fp32 -> out xt_tm [101, 131072] fp32."""
    import concourse.bacc as bacc
    import concourse.mybir as mybir
    import concourse.tile as tile

    F32 = mybir.dt.float32
    ALU = mybir.AluOpType
    ACTF = mybir.ActivationFunctionType

    nc = bacc.Bacc("TRN2", target_bir_lowering=False)
    u0_d = nc.dram_tensor("u0", [P, F], F32, kind="ExternalInput")
    out_d = nc.dram_tensor("xt_tm", [T, BC], F32, kind="ExternalOutput")

    nsteps = T - 1

    with tile.TileContext(nc) as tc:
        with (
            tc.tile_pool(name="state", bufs=2) as state_pool,
            tc.tile_pool(name="ph", bufs=3) as ph_pool,
            tc.tile_pool(name="sins", bufs=4) as sin_pool,
            tc.tile_pool(name="accs", bufs=3) as acc_pool,
            tc.tile_pool(name="outs", bufs=3) as out_pool,
        ):
            u = state_pool.tile([P, F], F32)
            nc.sync.dma_start(u[:], u0_d[:])

            def emit_wrapped_slice(u_tile, t_idx):
                # slice = 2pi * (u - round(u))
                q = out_pool.tile([P, F], F32, tag="q")
                nc.vector.tensor_scalar(
                    q[:], u_tile[:], MAGIC, MAGIC, op0=ALU.add, op1=ALU.subtract
                )
                m = out_pool.tile([P, F], F32, tag="m")
                nc.vector.scalar_tensor_tensor(
                    m[:], q[:], -1.0, u_tile[:], op0=ALU.mult, op1=ALU.add
                )
                s = out_pool.tile([P, F], F32, tag="slice")
                nc.vector.tensor_scalar(s[:], m[:], TWO_PI, None, op0=ALU.mult)
                nc.sync.dma_start(
                    out_d[t_idx].rearrange("(p f) -> p f", p=P), s[:]
                )

            emit_wrapped_slice(u, 0)

            for n in range(1, nsteps + 1):
                sins = []
                for k in range(1, FS_ORDER):
                    t_k = ph_pool.tile([P, F], F32, tag="t")
                    nc.vector.tensor_scalar(
                        t_k[:], u[:], float(k), Bk[k], op0=ALU.mult, op1=ALU.add
                    )
                    q_k = ph_pool.tile([P, F], F32, tag="q")
                    nc.vector.tensor_scalar(
                        q_k[:], t_k[:], MAGIC, MAGIC, op0=ALU.add, op1=ALU.subtract
                    )
                    m_k = ph_pool.tile([P, F], F32, tag="m")
                    nc.vector.scalar_tensor_tensor(
                        m_k[:], q_k[:], -1.0, t_k[:], op0=ALU.mult, op1=ALU.add
                    )
                    s_k = sin_pool.tile([P, F], F32)
                    nc.scalar.activation(s_k[:], m_k[:], ACTF.Sin, bias=0.0, scale=TWO_PI)
                    sins.append(s_k)

                # acc = c0 + sum_k R_k * s_k ; u' = u + acc
                a = acc_pool.tile([P, F], F32, tag="a")
                nc.vector.tensor_scalar(
                    a[:], sins[0][:], Rk[1], c0, op0=ALU.mult, op1=ALU.add
                )
                for k in range(2, FS_ORDER):
                    a2 = acc_pool.tile([P, F], F32, tag="a")
                    nc.vector.scalar_tensor_tensor(
                        a2[:], sins[k - 1][:], Rk[k], a[:], op0=ALU.mult, op1=ALU.add
                    )
                    a = a2
                u_new = state_pool.tile([P, F], F32)
                nc.vector.tensor_tensor(u_new[:], a[:], u[:], op=ALU.add)
                u = u_new

                emit_wrapped_slice(u, n)

    nc.compile()
    return nc


def kernel(x0_sample, sin_weight, cos_weight, t_sample):
    from concourse import bass_utils

    x0 = np.asarray(x0_sample, dtype=np.float32)
    a = np.asarray(sin_weight, dtype=np.float64)
    b = np.asarray(cos_weight, dtype=np.float64)

    # fold weights: a_k sin(kx) + b_k cos(kx) = r_k sin(kx + phi_k)
    Bk = {}
    Rk = {}
    for k in range(1, FS_ORDER):
        r = math.hypot(a[k], b[k])
        phi = math.atan2(b[k], a[k])
        Bk[k] = float(phi / TWO_PI)
        Rk[k] = float(DT * r / TWO_PI)
    c0 = float(DT * b[0] / TWO_PI)

    nc = _build_bass(Bk, Rk, c0)
    global _LAST_NC
    _LAST_NC = nc

    u0 = (x0.astype(np.float64) / TWO_PI).astype(np.float32)
    shards = u0.reshape(NCORES, P, F)
    in_maps = [{"u0": np.ascontiguousarray(shards[c])} for c in range(NCORES)]

    res = bass_utils.run_bass_kernel_spmd(nc, in_maps, core_ids=list(range(NCORES)))

    xt = np.empty((B, T), dtype=np.float32)
    for c in range(NCORES):
        xt[c * BC : (c + 1) * BC] = res.results[c]["xt_tm"].T

    t = np.arange(0.0, 2001.0, DT, dtype=np.float32)
    t_mesh = np.broadcast_to(t[None, :], (B, T))
    return (t_mesh, xt)


if __name__ == "__main__":
    # tiny self-check against a numpy reference
    rng = np.random.default_rng(0)
    x0 = rng.standard_normal(B).astype(np.float32)
    sw = (1e-4 / 8 * rng.standard_normal(8)).astype(np.float32)
    cw = (1e-4 / 8 * rng.standard_normal(8)).astype(np.float32)
    ts = rng.integers(0, 2000, B).astype(np.int32)
    tm, xt = kernel(x0, sw, cw, ts)
    print("xt", xt.shape, xt.dtype, xt[:2, :5])



# revision 4
# speedup vs baseline: 2.8876x; 2.8876x over previous
"""Trainium2 Bass kernel for nn_Drifter (Euler integration of Fourier drift ODE).

Reference semantics:
    t = arange(0, 2001, 20) (T=101 points)
    drift(x) = sin(x*orders) @ sin_w + cos(x*orders) @ cos_w   (orders 0..7)
    x_{n+1} = x_n + drift(x_n) * 20
    xt[B, T] = all iterates wrapped to [-pi, pi);  t_mesh = broadcast t.

Device algorithm (turns space u = x/2pi, batch sharded 8 ways, 2 column
groups per core for cross-step pipelining):
    drift*DT/2pi = c0 + sum_{k=1..7} R_k sin(2pi(k u + B_k))
    state v_n = u_n - n*c0 (constant drift folded into per-step immediates)
    per step, per group:
      m_k = t - round(t), t = k*v + frac(B_k + k n c0)   [one fused custom DVE
                                                          op per harmonic]
      s_k = Sin(2pi' m_k)  in waves, f32r output         [ACT]
      T   = sum_k R_k s_k  in PSUM                       [PE f32r matmuls; each
                            R_k split hi(10-bit exact) + lo so the amplitude
                            survives f32r weight truncation]
      v'  = v + T                                        [DVE stt, PSUM src]
    v_n is DMAed out time-major; host applies y = frac(v + n c0) (exact fp32,
    reference boundary convention) and the 2pi scale during the unshard
    transpose.
"""

import math

import numpy as np

B = 1048576
T = 101
T_STEPS = 100
NCORES = 8
BC = B // NCORES           # 131072 per core
P = 128
FT = BC // P               # 1024 free elements per partition per core
FS_ORDER = 8
NH = 7
DT = 20.0
TWO_PI = 2.0 * math.pi
MAGIC = 12582912.0         # 1.5 * 2**23: (x + M) - M == round-half-even(x)
SIN_SCALE = TWO_PI * (1.0 - 2.0 ** -22)  # keep spline arg strictly in [-pi, pi]

CFG = {
    "G": 2,
    "wave_sets": ((1,), (2, 3), (4, 5), (6, 7)),
    "merge": "f32r2",      # f32r2 | f32r1 | fp32
    "fp32_pe_set": (1, 2, 3, 4, 5, 6),
}

_LAST_NC = None
_op_cache = {}


def _get_custom_op():
    """Register (once) the fused phase+round+frac DVE op."""
    if "pf" in _op_cache:
        return _op_cache["pf"]
    import concourse.dve_ops as dve_ops
    from concourse.dve_spec import C0, C1, C2, Spec, Src0, lower
    from concourse.dve_uop import DveOpSpec

    name = "PHASE_FRAC_ANT"
    t = Src0 * C0 + C1
    q = (t + C2) - C2
    body = t - q

    def ref(in0, in1, s0, s1, imm2):
        tt = (in0 * np.float32(s0) + np.float32(s1)).astype(np.float32)
        qq = ((tt + np.float32(imm2)) - np.float32(imm2)).astype(np.float32)
        return (tt - qq).astype(np.float32)

    spec = Spec(body=body, reference=ref)
    existing = {op.name: op for op in dve_ops.OPS}
    if name in existing:
        _op_cache["pf"] = existing[name]
        return existing[name]
    row = dve_ops._CUSTOM_DVE_ROW_BASE + len(dve_ops.OPS)
    shas = {}
    for ver in ("v3", "v4"):
        tmp = DveOpSpec(name=name, opcode=row, uops=lower(spec, ver=ver), rd1_en=False)
        shas[ver] = tmp.sha(ver)
    op = dve_ops.DveOp(name, spec, subdim=False, uops_sha=shas)
    dve_ops.OPS.append(op)
    dve_ops.CUSTOM_DVE_SPECS[name] = spec
    dve_ops._SUB_OPCODE_FOR_NAME[name] = row
    _op_cache["pf"] = op
    return op


def _fold_weights(sin_weight, cos_weight):
    a = np.asarray(sin_weight, dtype=np.float64)
    b = np.asarray(cos_weight, dtype=np.float64)
    Rk = np.zeros(FS_ORDER)
    Bk = np.zeros(FS_ORDER)
    for k in range(1, FS_ORDER):
        Rk[k] = DT * math.hypot(a[k], b[k]) / TWO_PI
        Bk[k] = (math.atan2(b[k], a[k]) / TWO_PI) % 1.0
    c0 = DT * b[0] / TWO_PI
    return Rk, Bk, c0


def _trunc10(x):
    """Round x to 10 explicit mantissa bits (exact in any tf32-like format)."""
    f = np.float32(x)
    if f == 0:
        return f
    bits = f.view(np.int32)
    bits = np.int32(bits & ~np.int32((1 << 13) - 1))
    return bits.view(np.float32)


def _build_bass(Rk, Bk, c0):
    """Inputs: v0 [128, 1024] f32, wmat [128, NW*128] (f32r or f32).
    Output: y_tm [101, 131072] f32 — raw v states, time-major."""
    import concourse.bacc as bacc
    import concourse.mybir as mybir
    import concourse.tile as tile

    F32 = mybir.dt.float32
    F32R = mybir.dt.float32r
    ALU = mybir.AluOpType
    ACTF = mybir.ActivationFunctionType

    G = CFG["G"]
    Fg = FT // G
    wave_sets = CFG["wave_sets"]
    merge = CFG["merge"]
    f32r = merge.startswith("f32r")
    passes = 2 if merge == "f32r2" else 1
    if f32r:
        pe_set = set(range(1, FS_ORDER))
    else:
        pe_set = set(CFG["fp32_pe_set"])
    NW = passes * NH if f32r else len(pe_set)
    WD = F32R if f32r else F32

    pf_op = _get_custom_op()

    nc = bacc.Bacc("TRN2", target_bir_lowering=False)
    v0_d = nc.dram_tensor("v0", [P, FT], F32, kind="ExternalInput")
    w_d = nc.dram_tensor("wmat", [P, NW * P], WD, kind="ExternalInput")
    out_d = nc.dram_tensor("y_tm", [T, BC], F32, kind="ExternalOutput")

    # weight slot index per (harmonic, pass)
    wslot_of = {}
    si = 0
    for k in sorted(pe_set):
        for p_ in range(passes if f32r else 1):
            wslot_of[(k, p_)] = si
            si += 1

    def Bimm(k, n):
        return float((Bk[k] + k * ((n * c0) % 1.0)) % 1.0)

    with tile.TileContext(nc) as tc:
        with (
            tc.tile_pool(name="wpool", bufs=1) as wpool,
            tc.tile_pool(name="state", bufs=3) as state_pool,
            tc.tile_pool(name="mpool", bufs=2) as m_pool,
            tc.tile_pool(name="spool", bufs=2) as s_pool,
            tc.tile_pool(name="tpool", bufs=2) as t_pool,
            tc.tile_pool(name="psum", bufs=2, space="PSUM") as psum_pool,
        ):
            wtile = wpool.tile([P, NW, P], WD)
            nc.sync.dma_start(wtile[:], w_d[:].rearrange("p (h q) -> p h q", h=NW))

            v = []
            for g in range(G):
                vt = state_pool.tile([P, Fg], F32, tag=f"v{g}")
                nc.sync.dma_start(vt[:], v0_d[:, g * Fg:(g + 1) * Fg])
                v.append(vt[:])

            def emit_out(vap, g, n):
                dst = out_d[n].rearrange("(p f) -> p f", p=P)[:, g * Fg:(g + 1) * Fg]
                nc.sync.dma_start(dst, vap)

            for g in range(G):
                emit_out(v[g], g, 0)

            for n in range(T_STEPS):
                for g in range(G):
                    vt = v[g]
                    mw = {}
                    sw = {}
                    wslot = {}
                    for wi, wv in enumerate(wave_sets):
                        mtile = m_pool.tile(
                            [P, len(wv), Fg], F32, tag=f"m{g}w{wi}", name=f"m{g}w{wi}"
                        )
                        stile = s_pool.tile(
                            [P, len(wv), Fg], F32R if f32r else F32,
                            tag=f"s{g}w{wi}", name=f"s{g}w{wi}",
                        )
                        mw[wi] = mtile
                        sw[wi] = stile
                        for j, k in enumerate(wv):
                            wslot[k] = (wi, j)

                    # fused phase + magic-round + frac per harmonic (DVE)
                    for wv in wave_sets:
                        for k in wv:
                            wi, j = wslot[k]
                            nc.vector._custom_dve(
                                pf_op, out=mw[wi][:, j], in0=vt,
                                s0=float(k), s1=Bimm(k, n), imm2=MAGIC,
                            )

                    pt = psum_pool.tile([P, Fg], F32, tag=f"T{g}")
                    nodes = [vt, None]   # None -> placeholder for T
                    partials = []
                    dve_folds = []
                    pe_seen = 0
                    npe = len(pe_set) * passes

                    for wi, wv in enumerate(wave_sets):
                        nc.scalar.activation(
                            sw[wi][:], mw[wi][:], ACTF.Sin, bias=0.0, scale=SIN_SCALE
                        )
                        for k in wv:
                            j = wslot[k][1]
                            if k in pe_set:
                                for p_ in range(passes if f32r else 1):
                                    nc.tensor.matmul(
                                        pt[:], wtile[:, wslot_of[(k, p_)]], sw[wi][:, j],
                                        start=(pe_seen == 0),
                                        stop=(pe_seen == npe - 1),
                                    )
                                    pe_seen += 1
                            else:
                                dve_folds.append((k, sw[wi][:, j]))

                    # DVE folds for non-PE harmonics (fp32 fallback mode)
                    acc = None
                    ei = 0
                    for k, sap in dve_folds:
                        base = vt if acc is None else acc
                        dst = t_pool.tile(
                            [P, Fg], F32, tag=f"a{g}_{ei}", name=f"a{g}_{ei}"
                        )
                        ei += 1
                        nc.vector.scalar_tensor_tensor(
                            dst[:], sap, float(Rk[k]), base,
                            op0=ALU.mult, op1=ALU.add,
                        )
                        acc = dst[:]
                    base = vt if acc is None else acc

                    # v' = base + T  (DVE stt, PSUM source)
                    vn = t_pool.tile([P, Fg], F32, tag=f"vn{g}", name=f"vn{g}")
                    nc.vector.scalar_tensor_tensor(
                        vn[:], pt[:], 1.0, base, op0=ALU.mult, op1=ALU.add
                    )
                    v[g] = vn[:]

                    emit_out(v[g], g, n + 1)

    nc.compile()
    return nc


def _make_wmat(Rk):
    merge = CFG["merge"]
    f32r = merge.startswith("f32r")
    passes = 2 if merge == "f32r2" else 1
    pe_set = sorted(range(1, FS_ORDER)) if f32r else sorted(CFG["fp32_pe_set"])
    eye = np.eye(P, dtype=np.float32)
    blocks = []
    for k in pe_set:
        if f32r and passes == 2:
            hi = _trunc10(Rk[k])
            lo = np.float32(np.float64(Rk[k]) - np.float64(hi))
            blocks.append(eye * hi)
            blocks.append(eye * lo)
        else:
            blocks.append(eye * np.float32(Rk[k]))
    wmat = np.stack(blocks, axis=1)        # [P, NW, P]
    return np.ascontiguousarray(wmat.reshape(P, -1))


def kernel(x0_sample, sin_weight, cos_weight, t_sample):
    from concourse import bass_utils

    Rk, Bk, c0 = _fold_weights(sin_weight, cos_weight)
    nc = _build_bass(Rk, Bk, c0)
    global _LAST_NC
    _LAST_NC = nc

    x0 = np.asarray(x0_sample, dtype=np.float32)
    u0 = (x0.astype(np.float64) / TWO_PI).astype(np.float32)
    shards = u0.reshape(NCORES, P, FT)
    wmat = _make_wmat(Rk)
    in_maps = [
        {"v0": np.ascontiguousarray(shards[c]), "wmat": wmat} for c in range(NCORES)
    ]
    res = bass_utils.run_bass_kernel_spmd(nc, in_maps, core_ids=list(range(NCORES)))

    # host: y = frac_c(v + n*c0) with the reference wrap convention
    # (exact half-integers map to -0.5 i.e. -pi), then xt = 2pi * y^T.
    F = np.float32
    Cn = np.array([(n * c0) % 1.0 for n in range(T)], dtype=np.float32)[:, None]
    xt = np.empty((B, T), dtype=np.float32)
    for c in range(NCORES):
        v_tm = res.results[c]["y_tm"]                   # [T, BC] f32
        tp = (v_tm + Cn).astype(F)
        m = (tp - np.rint(tp)).astype(F)                # rint = round-half-even
        m = np.where(m == F(0.5), (m - F(1.0)).astype(F), m)
        xt[c * BC:(c + 1) * BC] = (m.T * F(TWO_PI)).astype(F)

    t = np.arange(0.0, 2001.0, DT, dtype=np.float32)
    t_mesh = np.broadcast_to(t[None, :], (B, T))
    return (t_mesh, xt)
